# revision 41
# baseline (speedup 1.0000x reference)
"""Bionetwork sparse-matvec recurrence on 8 trn2 NeuronCores.

y_{t+1} = act(A y_t + b_in), 150 iterations, A fixed sparse (3.2M edges,
100k nodes).  Dest-sharded across 8 cores; all routing tables SBUF-resident.

Per iteration, per core (local_scatter = vectorized GPSIMD within-row scatter):
  1. seed-scatter per dest-chunk g: canonical y -> run-starts of expansion
  2. segmented forward-fill via one tensor_tensor_scan (state=mask*state+seed)
  3. multiply by edge weights (fp16, in place)
  4. round-1 local_scatter: products -> staging tiles at col 128*t + dest_row
  5. PE transpose of each [128,128] staging tile (the cross-partition hop)
  6. round-2 local_scatter: transposed stream -> dest-slot layout
  7. segmented reduce straight into output order (per chunk: 15 64-wide
     slots for deg>32 dests + 18 32-wide slots; no fold pass)
  8. v = s + b_in; piecewise activation (select-fused); AllGather; reload y

Chunk assignment of dests is a greedy min-max balance (any dest fits any
chunk), flattening max edges per (chunk, src partition, dest row) -- that
max sets the staging-tile count T and the round-1/2 scatter cost.

Everything is table-driven; tables are built host-side from the (fixed)
edge lists and shipped as per-core input tensors to one shared program.
"""
import numpy as np

N = 100000
E = 3200000
P = 128
NCORES = 8
QW = 800                    # canonical width: 128*800 = 102400
NC_PAD = P * QW
SHARD = NC_PAD // NCORES    # 12800 = 128*100
KMAX = SHARD // P           # 100
ITERS = 150
LEAK = 0.01
RUN_CAP = 16                # fill rounds 1,2,4,8 cover runs of 16
SEED_REGIONS = 1
MAX_DST = 2046
TILES_PER_CALL = 15
SD = SEED_REGIONS * QW


def _ceil(a, b):
    return -(-a // b)


def _slot_of(c_src, d_core):
    """RDMA all-gather slot holding core c_src's shard on core d_core.

    Each core fires 7 single-dest relative remote_dma_broadcasts, slot x
    targeting Delta-tpb x.  Under the TRN2 physical-NC map (logical l -> real
    l for l<4, l^2 for l>=4) the slot-x write of sender s lands on core s^x
    for x<4 and s^x^2 for x>=4; inverting gives the receiver-side owner map.
    """
    delta = c_src ^ d_core
    return np.where(delta < 4, delta, delta ^ 2)


def _prep(x, in_weights, rec_weights, biases, out_weights,
          in_indices, edge_rows, edge_cols, out_indices):
    deg = np.bincount(edge_rows, minlength=N)
    assert deg.max() <= 64, f"max in-degree {deg.max()} > 64 unsupported"
    np2 = deg > 32  # wide dests get a 64-col slot, the rest a 32-col slot

    # Slot layout: every chunk holds NP2C 64-wide slots + NP1C 32-wide slots
    # per (core,row) bin, so ANY dest can be placed in ANY chunk.  A greedy
    # min-max pass then assigns dests to chunks to flatten the edge count per
    # (chunk, src partition, dest row) cell -- that max sets the staging tile
    # count T and with it the round-1/round-2 scatter cost.
    NP2C, NP1C = 15, 18
    NCH = 3
    CH = NP2C * 64 + NP1C * 32          # 1536
    FD = NCH * CH
    SLOTC = NP2C + NP1C                 # sp slots per chunk
    Kreal = KP = NCH * SLOTC            # 99
    assert Kreal <= KMAX
    NB = NCORES * P

    # deal dests round-robin over bins, np2 class first, LPT within class
    # (largest degree first improves the greedy min-max balance)
    rng = np.random.default_rng(12345)
    order = np.lexsort((rng.permutation(N), -deg, ~np2))
    i = np.arange(N)
    binid_pos = i % NB
    c_node = np.empty(N, np.int64)
    j_node = np.empty(N, np.int64)
    bin_node = np.empty(N, np.int64)
    c_node[order] = binid_pos % NCORES
    j_node[order] = binid_pos // NCORES
    bin_node[order] = binid_pos
    N2 = int(np2.sum())
    assert _ceil(N2, NB) <= NP2C * NCH and _ceil(N - N2, NB) <= NP1C * NCH
    rank_pos = np.empty(N, np.int64)
    rank_pos[order[:N2]] = np.arange(N2) // NB
    i1 = np.arange(N2, N)
    rank_pos[order[N2:]] = (i1 - N2 - ((i1 % NB) - N2) % NB) // NB
    r2max = int(rank_pos[order[:N2]].max()) + 1 if N2 else 0
    r1max = int(rank_pos[order[N2:]].max()) + 1

    # source-side fix: cap the per-(core,row,src-partition) edge TOTAL so the
    # dest-side chunk balance can land at MTg<=CAPT/3.  Swapping two
    # same-class nodes' dealing slots moves both their dest bin and their
    # source partition; accept swaps that lower the hot cell without pushing
    # any affected cell to the current max.  Delta-evaluated per candidate
    # (only edges incident to the swapped pair change cells).
    CAPT = 33
    rng2 = np.random.default_rng(777)
    ew_d, ew_s = edge_rows, edge_cols

    o_d = np.argsort(ew_d, kind="stable")
    dptr0 = np.searchsorted(ew_d[o_d], np.arange(N + 1))
    o_s = np.argsort(ew_s, kind="stable")
    sptr0 = np.searchsorted(ew_s[o_s], np.arange(N + 1))

    def _keys(eidx):
        return ((c_node[ew_d[eidx]] * P + j_node[ew_d[eidx]]) * P
                + j_node[ew_s[eidx]])

    def _tot():
        return np.bincount(_keys(np.arange(E)), minlength=NCORES * P * P)

    def _inc_edges(node):
        return np.concatenate([o_d[dptr0[node]:dptr0[node + 1]],
                               o_s[sptr0[node]:sptr0[node + 1]]])

    def _swap(a, bnode):
        for arr in (c_node, j_node, bin_node, rank_pos):
            arr[a], arr[bnode] = arr[bnode], arr[a]

    import time as _time
    tot = _tot()
    t_end = _time.time() + 60.0
    for _ in range(6000):
        cur = int(tot.max())
        if cur <= CAPT or _time.time() > t_end:
            break
        h = int(np.argmax(tot))
        hc, hj, hp = h // (P * P), (h // P) % P, h % P
        nodes_hj = np.flatnonzero((c_node == hc) & (j_node == hj))
        he = np.concatenate([o_d[dptr0[n]:dptr0[n + 1]] for n in nodes_hj])
        he = he[j_node[ew_s[he]] == hp]
        srcs, cnts = np.unique(ew_s[he], return_counts=True)
        s = int(srcs[np.argmax(cnts)])
        pool_t = np.flatnonzero(np2 == np2[s])
        es_ = _inc_edges(s)
        best = None
        for t in rng2.choice(pool_t, size=48, replace=False):
            t = int(t)
            if t == s:
                continue
            eidx = np.concatenate([es_, _inc_edges(t)])
            k_old = _keys(eidx)
            _swap(s, t)
            k_new = _keys(eidx)
            _swap(s, t)
            aff = np.unique(np.concatenate([k_old, k_new]))
            av = tot[aff].astype(np.int64)
            np.subtract.at(av, np.searchsorted(aff, k_old), 1)
            np.add.at(av, np.searchsorted(aff, k_new), 1)
            hpos = np.searchsorted(aff, h)
            if hpos >= aff.size or aff[hpos] != h or av[hpos] >= cur:
                continue
            nm = int(av.max())
            if nm >= cur:
                continue
            sc = (nm, int((av > CAPT).sum()))
            if best is None or sc < best[:2]:
                best = (nm, sc[1], t, k_old, k_new)
        if best is None:
            continue
        _, _, tbest, k_old, k_new = best
        eidx = np.concatenate([es_, _inc_edges(tbest)])
        np.subtract.at(tot, k_old, 1)
        _swap(s, tbest)
        np.add.at(tot, _keys(eidx), 1)
    # greedy chunk assignment, one round per (class, rank): each bin places
    # its rank-r dest into the chunk minimizing that bin-row's max cell
    p0_of_node = j_node
    e_b = bin_node[edge_rows]
    e_p0 = p0_of_node[edge_cols]
    e_key = np.where(np2[edge_rows], 0, 256) + rank_pos[edge_rows]
    eo = np.argsort(e_key, kind="stable")
    e_key_s = e_key[eo]
    n_key = np.where(np2, 0, 256) + rank_pos
    no = np.argsort(n_key, kind="stable")
    n_key_s = n_key[no]

    cells = np.zeros((NCH, NB, P), np.int32)
    cnt2 = np.zeros((NCH, NB), np.int32)
    cnt1 = np.zeros((NCH, NB), np.int32)
    band_of = np.zeros(N, np.int64)
    kloc_of = np.zeros(N, np.int64)

    def _rounds(base_key, nmax, cnt, cap):
        for r in range(nmax):
            key = base_key + r
            na, nb_ = np.searchsorted(n_key_s, [key, key + 1])
            if na == nb_:
                continue
            nodes_r = no[na:nb_]
            ea, ebnd = np.searchsorted(e_key_s, [key, key + 1])
            H = np.zeros((NB, P), np.int32)
            if ea < ebnd:
                es = eo[ea:ebnd]
                np.add.at(H, (e_b[es], e_p0[es]), 1)
            cmax = (cells + H[None]).max(axis=2) * 64 + cnt
            cmax[cnt >= cap] = 1 << 30
            band = np.argmin(cmax, axis=0)
            bsel = bin_node[nodes_r]
            bb = band[bsel]
            band_of[nodes_r] = bb
            kloc_of[nodes_r] = cnt[bb, bsel]
            for ch in range(NCH):
                m = bsel[bb == ch]
                cells[ch, m] += H[m]
                cnt[ch, m] += 1

    _rounds(0, r2max, cnt2, NP2C)
    _rounds(256, r1max, cnt1, NP1C)

    # swap-refinement: the greedy's per-chunk max cells are online-placement
    # flukes; swapping two same-class dests of one bin between chunks just
    # exchanges their (chunk, slot) pairs, so descend on the hot cells
    eds = np.argsort(edge_rows, kind="stable")
    er_s = edge_rows[eds]
    dptr = np.searchsorted(er_s, np.arange(N + 1))
    e_p0s = e_p0[eds]

    def _hist(d):
        return np.bincount(e_p0s[dptr[d]:dptr[d + 1]], minlength=P)

    bysort = np.lexsort((np.arange(N), bin_node))
    bptr = np.searchsorted(bin_node[bysort], np.arange(NB + 1))
    t_end2 = _time.time() + 60.0
    stall = 0
    while True:
        m = int(cells.max())
        if m <= 11 or _time.time() > t_end2 or stall > 30:
            break
        hot = np.argwhere(cells == m)
        progressed = False
        for ch, bin_, p0h in hot[:16]:
            ch = int(ch)
            dl = bysort[bptr[bin_]:bptr[bin_ + 1]]
            mine = dl[band_of[dl] == ch]
            hists = {int(d): _hist(int(d)) for d in dl}
            best = None
            for d in mine:
                hd = hists[int(d)]
                if hd[p0h] == 0:
                    continue
                for d2 in dl[(band_of[dl] != ch) & (np2[dl] == np2[d])]:
                    ch2 = int(band_of[d2])
                    h2 = hists[int(d2)]
                    na = int((cells[ch, bin_] - hd + h2).max())
                    nb_ = int((cells[ch2, bin_] - h2 + hd).max())
                    # plateau swaps (nb_ == m) allowed: they move the hot
                    # cell even when a strict improvement doesn't exist
                    if na < m and nb_ <= m:
                        sc = (max(na, nb_), na + nb_)
                        if best is None or sc < best[:2]:
                            best = (sc[0], sc[1], int(d), d2, ch2)
            if best is not None:
                _, _, d, d2, ch2 = best
                hd, h2 = hists[d], hists[int(d2)]
                cells[ch, bin_] += h2 - hd
                cells[ch2, bin_] += hd - h2
                band_of[d], band_of[d2] = ch2, ch
                kloc_of[d], kloc_of[d2] = kloc_of[d2], kloc_of[d]
                progressed = True
        stall = 0 if progressed else stall + 1

    k_node = band_of * SLOTC + np.where(np2, kloc_of, NP2C + kloc_of)
    perm = SHARD * c_node + KMAX * j_node + k_node

    # per-edge slot column: rank within dest (stable edge order)
    so = np.argsort(edge_rows, kind="stable")
    ds = edge_rows[so]
    st = np.r_[0, np.flatnonzero(np.diff(ds)) + 1]
    sid = np.zeros(E, np.int64)
    sid[st[1:]] = 1
    sid = np.cumsum(sid)
    e_drank = np.empty(E, np.int64)
    e_drank[so] = np.arange(E) - st[sid]
    e_np2 = np2[edge_rows]
    e_kloc = kloc_of[edge_rows]
    f_local = np.where(e_np2, 64 * e_kloc + e_drank,
                       NP2C * 64 + 32 * e_kloc + e_drank)
    fglob = band_of[edge_rows] * CH + f_local

    import jax.numpy as jnp
    node_in = np.asarray(
        jnp.zeros((N,), jnp.float32).at[jnp.asarray(in_indices)].set(
            jnp.asarray(in_weights, jnp.float32) * jnp.asarray(x[0], jnp.float32)))
    b_in_full = node_in + biases.astype(np.float32)

    dnew, snew = perm[edge_rows], perm[edge_cols]
    w_all = rec_weights.astype(np.float32)
    dcore = dnew // SHARD

    # ---------- pass 1: per-core edge geometry ----------
    # per-core y layout: y_parts[j, 100*slot_of(c_src, core) + k] holds the
    # (c_src, j, k) shard entry (each slot is the sender's y16 tile verbatim)
    geo = []
    for c in range(NCORES):
        em = np.where(dcore == c)[0]
        d_loc = dnew[em] - SHARD * c
        j, k = d_loc // KMAX, d_loc % KMAX
        s_new = snew[em]
        s_c, s_jk = s_new // SHARD, s_new % SHARD
        p0 = s_jk // KMAX
        q0 = KMAX * _slot_of(s_c, c) + s_jk % KMAX
        w = w_all[em]
        ne = em.size

        def ranks_of(key):
            so = np.argsort(key, kind="stable")
            ks = key[so]
            st = np.r_[0, np.flatnonzero(np.diff(ks)) + 1]
            sid = np.zeros(ne, np.int64)
            sid[st[1:]] = 1
            sid = np.cumsum(sid)
            r = np.arange(ne) - st[sid]
            out = np.empty(ne, np.int64)
            out[so] = r
            return out

        f = fglob[em]
        g = f // CH
        trank = ranks_of((g * P + p0) * P + j)
        # expansion position within (g,p0) ordered by q0, and rank within source
        so3 = np.lexsort((q0, p0, g))
        gp = (g * P + p0)[so3]
        st = np.r_[0, np.flatnonzero(np.diff(gp)) + 1]
        sid = np.zeros(ne, np.int64)
        sid[st[1:]] = 1
        sid = np.cumsum(sid)
        m_pos = np.empty(ne, np.int64)
        m_pos[so3] = np.arange(ne) - st[sid]
        gpq = ((g * P + p0) * QW + q0)[so3]
        st4 = np.r_[0, np.flatnonzero(np.diff(gpq)) + 1]
        sid4 = np.zeros(ne, np.int64)
        sid4[st4[1:]] = 1
        sid4 = np.cumsum(sid4)
        src_rank = np.empty(ne, np.int64)
        src_rank[so3] = np.arange(ne) - st4[sid4]
        geo.append(dict(j=j, p0=p0, q0=q0, w=w, f=f, g=g,
                        trank=trank, m_pos=m_pos, src_rank=src_rank, ne=ne))

    # uniform per-chunk sizes across cores
    M1 = np.zeros(NCH, np.int64)
    MTg = np.zeros(NCH, np.int64)
    for gg in geo:
        for g2 in range(NCH):
            sel = gg["g"] == g2
            if sel.any():
                M1[g2] = max(M1[g2], int(gg["m_pos"][sel].max()) + 1)
                MTg[g2] = max(MTg[g2], int(gg["trank"][sel].max()) + 1)
    M1 = (_ceil(M1, 2) * 2).astype(np.int64)
    EB = np.r_[0, np.cumsum(M1)]         # expansion bases
    MEXP = int(EB[-1])
    TBASE = np.r_[0, np.cumsum(MTg)]     # tile bases
    T = int(TBASE[-1])
    # round-1 call structure: (g, t0, t1), evenly-split windows <= 15 tiles
    r1_struct = []
    for g2 in range(NCH):
        tg = int(MTg[g2])
        ncall = _ceil(tg, TILES_PER_CALL)
        base, rem = divmod(tg, ncall)
        t0 = 0
        for ci in range(ncall):
            nt = base + (1 if ci < rem else 0)
            r1_struct.append((g2, t0, t0 + nt))
            t0 += nt
    NR1 = len(r1_struct)

    # ---------- pass 2: tables ----------
    cores = []
    for c in range(NCORES):
        gg = geo[c]
        j, p0, q0, w = gg["j"], gg["p0"], gg["q0"], gg["w"]
        f, g, trank, m_pos, src_rank = (gg["f"], gg["g"], gg["trank"],
                                        gg["m_pos"], gg["src_rank"])
        m_glob = EB[g] + m_pos
        dist = src_rank

        seedidx = np.full((NCH, P, SD), -1, np.int16)
        sm = dist == 0
        seedidx[g[sm], p0[sm], q0[sm]] = m_pos[sm].astype(np.int16)

        # scan fill mask: 1.0 inside a source run (copy state), 0.0 at starts
        runmask = np.zeros((P, MEXP), np.float16)
        mm = dist > 0
        runmask[p0[mm], m_glob[mm]] = 1.0

        w_exp = np.zeros((P, MEXP), np.float16)
        w_exp[p0, m_glob] = w.astype(np.float16)

        idx1 = []
        for (g2, t0, t1) in r1_struct:
            sel = (g == g2) & (trank >= t0) & (trank < t1)
            idx = np.full((P, int(M1[g2])), -1, np.int16)
            idx[p0[sel], m_pos[sel]] = (128 * (trank[sel] - t0) + j[sel]).astype(np.int16)
            idx1.append(idx)

        idx2 = []
        for g2 in range(NCH):
            sel = g == g2
            idx = np.full((P, 128 * int(MTg[g2])), -1, np.int16)
            idx[j[sel], 128 * trank[sel] + p0[sel]] = (f[sel] - g2 * CH).astype(np.int16)
            idx2.append(idx)

        b_in_t = np.zeros((P, Kreal), np.float32)
        nid = np.where((perm >= SHARD * c) & (perm < SHARD * (c + 1)))[0]
        dl = perm[nid] - SHARD * c
        b_in_t[dl // KMAX, dl % KMAX] = b_in_full[nid]

        cores.append(dict(seedidx=seedidx, runmask=runmask, w_exp=w_exp,
                          idx1=idx1, idx2=idx2, b_in_t=b_in_t))

    meta = dict(Kreal=Kreal, KP=KP, FD=FD, NCH=NCH, CH=CH, M1=M1, EB=EB,
                MTg=MTg, TBASE=TBASE, T=T, MEXP=MEXP, NR1=NR1,
                r1_struct=r1_struct, NP2C=NP2C, NP1C=NP1C, SLOTC=SLOTC)
    return cores, perm, meta


def _act_np(v):
    y1 = np.maximum(v, np.float32(LEAK) * v)
    ysat = (1.0 - 0.25 / np.maximum(v, 0.5)).astype(v.dtype)
    return np.where(v > 0.5, ysat, y1)


def _sim(cores, perm, meta, n_iters, quant=True):
    dt = np.float16 if quant else np.float32
    Kreal, KP, FD, NCH, CH = (meta["Kreal"], meta["KP"], meta["FD"],
                              meta["NCH"], meta["CH"])
    M1, EB, MTg, TBASE, T, MEXP = (meta["M1"], meta["EB"], meta["MTg"],
                                   meta["TBASE"], meta["T"], meta["MEXP"])
    y = np.zeros(NC_PAD, np.float32)
    jj_, kk_ = np.meshgrid(np.arange(P), np.arange(KMAX), indexing="ij")
    for it in range(n_iters):
        y_next = np.zeros(NC_PAD, np.float32)
        for c, tb in enumerate(cores):
            # per-core y_parts layout: slot x holds the shard of owner(c, x)
            seed_data = np.zeros((P, QW), dt)
            for x in range(NCORES):
                own = (c ^ x) if x < 4 else (c ^ x ^ 2)
                seed_data[:, KMAX * x:KMAX * (x + 1)] = y[
                    SHARD * own + KMAX * jj_ + kk_].astype(dt)
            seeds = np.zeros((P, MEXP), dt)
            for g2 in range(NCH):
                sidx = tb["seedidx"][g2]
                pp, cc = np.where(sidx >= 0)
                seeds[pp, EB[g2] + sidx[pp, cc]] = seed_data[pp, cc]
            # segmented forward-fill scan: state = mask*state + seed (fp32
            # state, downcast per element) per chunk
            exp_t = np.zeros((P, MEXP), dt)
            rm = tb["runmask"].astype(np.float32)
            sd32 = seeds.astype(np.float32)
            for g2 in range(NCH):
                st = np.zeros(P, np.float32)
                for t in range(int(EB[g2]), int(EB[g2 + 1])):
                    st = rm[:, t] * st + sd32[:, t]
                    exp_t[:, t] = st.astype(dt)
            prod = (exp_t.astype(np.float32) * tb["w_exp"].astype(np.float32)).astype(dt)
            staging = np.zeros((P, 128 * T), dt)
            for ci, (g2, t0, t1) in enumerate(meta["r1_struct"]):
                idx = tb["idx1"][ci]
                data = prod[:, EB[g2]:EB[g2] + M1[g2]]
                pp, cc = np.where(idx >= 0)
                staging[pp, 128 * (TBASE[g2] + t0) + idx[pp, cc]] = data[pp, cc]
            t2 = np.zeros_like(staging)
            for t in range(T):
                t2[:, 128 * t:128 * (t + 1)] = staging[:, 128 * t:128 * (t + 1)].T
            slots = np.zeros((P, FD), dt)
            for g2 in range(NCH):
                idx = tb["idx2"][g2]
                data = t2[:, 128 * TBASE[g2]:128 * (TBASE[g2] + MTg[g2])]
                pp, cc = np.where(idx >= 0)
                slots[pp, g2 * CH + idx[pp, cc]] = data[pp, cc]
            NP2C, NP1C, SLOTC = meta["NP2C"], meta["NP1C"], meta["SLOTC"]
            sp = np.zeros((P, KP), np.float32)
            for g2 in range(NCH):
                ch = slots[:, g2 * CH:(g2 + 1) * CH].astype(np.float32)
                w2 = ch[:, :NP2C * 64].reshape(P, NP2C, 64).sum(axis=2)
                w1 = ch[:, NP2C * 64:].reshape(P, NP1C, 32).sum(axis=2)
                c0 = g2 * SLOTC
                sp[:, c0:c0 + NP2C] = w2
                sp[:, c0 + NP2C:c0 + SLOTC] = w1
            s = sp.astype(dt).astype(np.float32)[:, :Kreal]
            v = s + tb["b_in_t"]
            y32 = _act_np(v)
            jj, kk2 = np.meshgrid(np.arange(P), np.arange(Kreal), indexing="ij")
            y_next[SHARD * c + KMAX * jj.ravel() + kk2.ravel()] = y32.ravel()
        y = y_next
    return y


# ============================ BASS KERNEL ============================

def _build(cores, meta, n_iters, no_cc=False, skip_last_exchange=True):
    import concourse.bacc as bacc
    import concourse.mybir as mybir
    import concourse.tile as tile
    from concourse.masks import make_identity

    f16, f32, i16 = mybir.dt.float16, mybir.dt.float32, mybir.dt.int16
    AOP = mybir.AluOpType
    Kreal, KP, FD, NCH, CH = (meta["Kreal"], meta["KP"], meta["FD"],
                              meta["NCH"], meta["CH"])
    M1, EB, MTg, TBASE, T, MEXP, NR1 = (meta["M1"], meta["EB"], meta["MTg"],
                                        meta["TBASE"], meta["T"],
                                        meta["MEXP"], meta["NR1"])
    NP2C, NP1C, SLOTC = meta["NP2C"], meta["NP1C"], meta["SLOTC"]
    DSTW = [min(FD, (g + 1) * CH) - g * CH for g in range(NCH)]

    nc = bacc.Bacc("TRN2", target_bir_lowering=False, num_swdge_queues=2)

    d_seed = [nc.dram_tensor(f"t_seed{g}", [P, SD], i16, kind="ExternalInput")
              for g in range(NCH)]
    d_rmask = nc.dram_tensor("t_rmask", [P, MEXP], f16, kind="ExternalInput")
    d_wexp = nc.dram_tensor("t_wexp", [P, MEXP], f16, kind="ExternalInput")
    d_idx1 = [nc.dram_tensor(f"t_idx1_{ci}", [P, int(M1[g2])], i16,
                             kind="ExternalInput")
              for ci, (g2, _, _) in enumerate(meta["r1_struct"])]
    d_idx2 = [nc.dram_tensor(f"t_idx2_{g}", [P, 128 * int(MTg[g])], i16,
                             kind="ExternalInput") for g in range(NCH)]
    d_bin = nc.dram_tensor("t_bin", [P, Kreal], f32, kind="ExternalInput")
    d_yout = nc.dram_tensor("y_out", [P, Kreal], f16, kind="ExternalOutput")

    # cross-core y exchange: 7 single-dest relative remote_dma_broadcasts per
    # iteration land each core's y16 in the peers' y_parts slots (slot order
    # [own,^1,^2,^3,^6,^7,^4,^5] = _slot_of under the Q7 lane map); rsem
    # counts arrivals (2/send), lsem send completions (16/send)
    rsem = nc.alloc_semaphore("rdma_rsem")
    lsem = nc.alloc_semaphore("rdma_lsem")
    patches = {}  # inst name -> (sem num, wait value); placeholder 0 -> value

    def _gate(inst, sem, val):
        inst.wait_op(sem, 0, "sem-ge")
        patches[inst.ins.name] = (sem.num, val)
        return inst

    with tile.TileContext(nc) as tc:
        with tc.tile_pool(name="tables", bufs=1) as tp, \
             tc.tile_pool(name="psum", bufs=8, space="PSUM") as pp:
            t_seed = [tp.tile([P, SD], i16, name=f"seed{g}") for g in range(NCH)]
            t_rmask = tp.tile([P, MEXP], f16, name="rmask")
            t_wexp = tp.tile([P, MEXP], f16, name="wexp")
            t_idx1 = [tp.tile([P, int(M1[g2])], i16, name=f"i1_{ci}")
                      for ci, (g2, _, _) in enumerate(meta["r1_struct"])]
            t_idx2 = [tp.tile([P, 128 * int(MTg[g])], i16, name=f"i2_{g}")
                      for g in range(NCH)]
            t_bin = tp.tile([P, Kreal], f32, name="bin")
            ident = tp.tile([P, P], f16, name="ident")
            ypA = tp.tile([P, QW], f16, name="ypA")
            ypB = tp.tile([P, QW], f16, name="ypB")
            ttok = tp.tile([1, 2], f16, name="ttok")
            expb = [tp.tile([P, int(M1[g])], f16, name=f"expb{g}")
                    for g in range(NCH)]
            seedb = [tp.tile([P, int(M1[g])], f16, name=f"seedb{g}")
                     for g in range(NCH)]
            stag = [tp.tile([P, 128 * int(MTg[g])], f16, name=f"stag{g}")
                    for g in range(NCH)]
            t2d = [tp.tile([P, 128 * int(MTg[g])], f16, name=f"t2d{g}")
                   for g in range(NCH)]
            slots = [tp.tile([P, DSTW[g]], f16, name=f"slots{g}")
                     for g in range(NCH)]
            sp = tp.tile([P, KP], f16, name="sp")
            vv = tp.tile([P, Kreal], f32, name="vv")
            y1b = tp.tile([P, Kreal], f32, name="y1b")
            rb = tp.tile([P, Kreal], f32, name="rb")
            y16 = tp.tile([P, KMAX], f16, name="y16")

            for g in range(NCH):
                nc.sync.dma_start(t_seed[g][:], d_seed[g][:])
                nc.sync.dma_start(t_idx2[g][:], d_idx2[g][:])
            nc.sync.dma_start(t_rmask[:], d_rmask[:])
            for ci in range(NR1):
                nc.sync.dma_start(t_idx1[ci][:], d_idx1[ci][:])
            nc.sync.dma_start(t_wexp[:], d_wexp[:])
            nc.sync.dma_start(t_bin[:], d_bin[:])
            make_identity(nc, ident[:])
            nc.vector.memset(ypA[:], 0.0)
            nc.vector.memset(y16[:], 0.0)

            r1_by_g = {}
            for ci, (g2, t0, t1) in enumerate(meta["r1_struct"]):
                r1_by_g.setdefault(g2, []).append((ci, t0, t1))

            # small chunk last: its short r1->copy->r2 chain ends the iteration
            g_order = [1, 0, 2] if NCH == 3 else list(range(NCH))

            def body(it, last=False):
                buf_r = ypA if it % 2 == 0 else ypB
                buf_w = ypB if it % 2 == 0 else ypA
                # phase 1: seeds, scan-fill, weight mult, round-1 scatters --
                # all chunks' r1 calls queue on Pool ahead of any r2, so the
                # last chunk's staging is ready before Pool reaches its r2
                for g in g_order:
                    w0, w1 = int(EB[g]), int(EB[g + 1])
                    mw = int(M1[g])
                    sc = nc.gpsimd.local_scatter(
                        seedb[g][:], buf_r[:], t_seed[g][:],
                        channels=P, num_elems=mw, num_idxs=SD)
                    if it > 0:
                        _gate(sc, rsem, 14 * it)
                    nc.vector.tensor_tensor_scan(
                        expb[g][:], t_rmask[:, w0:w1], seedb[g][:], 0.0,
                        op0=AOP.mult, op1=AOP.add)
                    nc.vector.tensor_tensor(expb[g][:], expb[g][:],
                                            t_wexp[:, w0:w1], op=AOP.mult)
                    for ci, t0, t1 in r1_by_g[g]:
                        nt = t1 - t0
                        nc.gpsimd.local_scatter(
                            stag[g][:, 128 * t0:128 * t1], expb[g][:],
                            t_idx1[ci][:], channels=P, num_elems=128 * nt,
                            num_idxs=mw)

                # phase 2: transposes; PSUM->SBUF copies alternate DVE/Act
                nbatch = 0
                for g in g_order:
                    Tg = int(MTg[g])
                    for tb0 in range(0, Tg, 8):
                        nb = min(8, Tg - tb0)
                        pt = pp.tile([P, 8 * P], f16, space="PSUM", tag="tr",
                                     name="tr")
                        for t in range(tb0, tb0 + nb):
                            nc.tensor.transpose(
                                pt[:, 128 * (t - tb0):128 * (t - tb0 + 1)],
                                stag[g][:, 128 * t:128 * (t + 1)], ident[:])
                        dst = t2d[g][:, 128 * tb0:128 * (tb0 + nb)]
                        if nbatch % 2 == 0:
                            nc.vector.tensor_copy(dst, pt[:, 0:128 * nb])
                        else:
                            nc.scalar.copy(dst, pt[:, 0:128 * nb])
                        nbatch += 1
                # phase 3: round-2 scatters + segmented reduces (64-wide
                # slots for wide dests then 32-wide; fp16 sums of |w*y|<0.2
                # stay O(1), validated against an fp64 reference)
                for g in g_order:
                    nc.gpsimd.local_scatter(
                        slots[g][:], t2d[g][:],
                        t_idx2[g][:], channels=P, num_elems=DSTW[g],
                        num_idxs=128 * int(MTg[g]))
                    c0 = g * SLOTC
                    n2w = NP2C * 64
                    with nc.allow_low_precision(reason="fp16 slot sums"):
                        nc.vector.tensor_reduce(
                            sp[:, c0:c0 + NP2C],
                            slots[g][:, 0:n2w].rearrange(
                                "p (k s) -> p k s", s=64),
                            axis=mybir.AxisListType.X, op=AOP.add)
                        nc.vector.tensor_reduce(
                            sp[:, c0 + NP2C:c0 + SLOTC],
                            slots[g][:, n2w:CH].rearrange(
                                "p (k s) -> p k s", s=32),
                            axis=mybir.AxisListType.X, op=AOP.add)
                # piecewise activation == min(leaky, saturating): for v<=0.5
                # the clamp makes 1-0.25/max(v,0.5) = 0.5 >= leaky(v); for
                # v>0.5 the saturating branch is always below v.  Sliced in
                # two: cols [0:66] (chunks 1,0, reduced early) run under the
                # last chunk's r2; only 33 cols remain on the tail chain.
                for a0, a1 in ((0, 2 * SLOTC), (2 * SLOTC, Kreal)):
                    nc.vector.tensor_tensor(vv[:, a0:a1], sp[:, a0:a1],
                                            t_bin[:, a0:a1], op=AOP.add)
                    nc.vector.scalar_tensor_tensor(
                        y1b[:, a0:a1], vv[:, a0:a1], float(LEAK), vv[:, a0:a1],
                        op0=AOP.mult, op1=AOP.max)
                    nc.vector.tensor_scalar_max(rb[:, a0:a1], vv[:, a0:a1], 0.5)
                    nc.vector.reciprocal(rb[:, a0:a1], rb[:, a0:a1])
                    nc.vector.tensor_scalar(rb[:, a0:a1], rb[:, a0:a1],
                                            -0.25, 1.0,
                                            op0=AOP.mult, op1=AOP.add)
                    ymin = nc.vector.tensor_tensor(y16[:, a0:a1], y1b[:, a0:a1],
                                                   rb[:, a0:a1], op=AOP.min)
                    if it > 0:
                        # previous round's sends must have finished reading y16
                        _gate(ymin, lsem, 112 * it)
                if last:
                    return  # final shard never leaves this core pre-gather
                # fire the prepared sends; the signals_writable token orders
                # next round's seeds after the trigger so their rsem gate is
                # reachable.  The local slot-0 copy comes AFTER the trigger:
                # tile's whole-tile dep granularity would otherwise chain
                # desc-gen behind copy behind activation.
                nc.gpsimd.trigger_dma(count=None,
                                      signals_writable=[buf_w[:, 0:2]])
                nc.scalar.copy(buf_w[:, 0:KMAX], y16[:])

            for it in range(n_iters):
                body(it, last=(skip_last_exchange and it == n_iters - 1))
            nc.sync.dma_start(d_yout[:], y16[:, 0:Kreal])

    # patch the placeholder cross-core gates to their real per-round values
    # (a real value would deadlock tile's single-core scheduling simulator)
    n_patched = 0
    for bb in nc.m.functions[0].blocks:
        for ins in bb.instructions:
            tgt = patches.get(ins.name)
            if tgt is None:
                continue
            si = ins.sync_info
            for w in (si.on_wait if si else []):
                if w.id == tgt[0] and w.wait_mode == "sem-ge-imm" \
                        and w.wait_value == 0:
                    w.wait_value = tgt[1]
                    n_patched += 1
    assert n_patched == len(patches), (n_patched, len(patches))

    nc.compile()
    return nc


def _in_maps(cores, meta):
    maps = []
    for tb in cores:
        m = {"t_wexp": tb["w_exp"], "t_bin": tb["b_in_t"],
             "t_rmask": tb["runmask"]}
        for g in range(meta["NCH"]):
            m[f"t_seed{g}"] = tb["seedidx"][g]
            m[f"t_idx2_{g}"] = tb["idx2"][g]
        for ci in range(meta["NR1"]):
            m[f"t_idx1_{ci}"] = tb["idx1"][ci]
        maps.append(m)
    return maps


def make_timing_cost_model():
    """Cost model for single-core TimelineSim (no_exec) runs of this kernel.

    The stock rust model has a known gap: in no_exec mode nobody enqueues
    remote-DMA prep entries into the SWDGE FIFO, so InstTriggerDma models no
    transfer time and no sem delivery, and the cross-core gates deadlock.
    This shim enqueues the prep entries exactly as the executor would (the
    rust trigger visit then applies its own transfer/ack formulas), and
    mirrors each incoming remote-sem delivery locally: in symmetric SPMD the
    peers' sends to this core complete at the same relative times as this
    core's sends to them.
    """
    import concourse.bass_isa as bass_isa
    import concourse.mybir as mb
    from concourse.cost_model import InstructionCostModel
    from concourse.cost_model_rust import (
        SemUpdate as EvSemUpdate, RemoteSemUpdate as EvRemoteSemUpdate)
    from concourse.dge_state import SwdgePrepEntry, return_none
    from concourse.hw_specs import get_hw_spec

    class RdmaCostModel(InstructionCostModel):
        def __init__(self, hw_spec):
            super().__init__(hw_spec)
            self._pending_mirror = {}  # queue_num -> [(sem_id, inc, n_dests)]

        def visit(self, instruction, sim):
            if sim.instruction_executor is not None:
                return super().visit(instruction, sim)
            if isinstance(instruction, bass_isa.InstRemoteDMABroadcastDescs):
                fifo = sim.swdge[instruction.queue_num]
                grp, ndescs = fifo.start_broadcast_group(instruction)
                fifo.await_space(SwdgePrepEntry(
                    instruction, ndescs, ndescs, instruction.local_sem_update,
                    return_none, grp))
                n_dests = len([d for d in instruction.dests if d >= 0])
                self._pending_mirror.setdefault(instruction.queue_num, []).append(
                    (instruction.remote_sem, 16 // len(instruction.dests),
                     n_dests))
            timelines = super().visit(instruction, sim)
            if isinstance(instruction, bass_isa.InstTriggerDma):
                pend = self._pending_mirror.get(instruction.queue_num, [])
                if pend:
                    out = []
                    for ev in timelines[0]:
                        out.append(ev)
                        if isinstance(ev, EvRemoteSemUpdate) and pend:
                            sem_id, inc, n_dests = pend[0]
                            out.append(EvSemUpdate(mb.SyncUpdate(
                                sync_type="semaphore", id=sem_id,
                                update_mode="sem-add-imm", update_value=inc)))
                            if n_dests <= 1:
                                pend.pop(0)
                            else:
                                pend[0] = (sem_id, inc, n_dests - 1)
                    timelines[0] = out
            return timelines

    return RdmaCostModel(get_hw_spec("TRN2"))


def _gather_y(res, meta):
    Kreal = meta["Kreal"]
    y_full = np.zeros(NC_PAD, np.float32)
    jj, kk2 = np.meshgrid(np.arange(P), np.arange(Kreal), indexing="ij")
    for c in range(NCORES):
        y32 = res.results[c]["y_out"]
        y_full[SHARD * c + KMAX * jj.ravel() + kk2.ravel()] = y32.ravel()
    return y_full


def kernel(**inputs):
    from concourse.bass_utils import run_bass_kernel_spmd
    inputs = {k: np.asarray(v) for k, v in inputs.items()}
    cores, perm, meta = _prep(**inputs)
    nc = _build(cores, meta, ITERS)
    maps = _in_maps(cores, meta)
    res = run_bass_kernel_spmd(nc, maps, core_ids=list(range(NCORES)))
    y_old = _gather_y(res, meta)[perm]
    out = (inputs["out_weights"].astype(np.float32)
           * y_old[inputs["out_indices"]])[None, :]
    return out.astype(np.float32)


if __name__ == "__main__":
    import sys, time
    sys.path.insert(0, "/root/problem")
    import reference
    inputs = {k: np.asarray(v) for k, v in reference.setup_inputs().items()}
    t0 = time.time()
    cores, perm, meta = _prep(**inputs)
    print(f"prep {time.time()-t0:.1f}s Kreal={meta['Kreal']} KP={meta['KP']} "
          f"FD={meta['FD']} M1={meta['M1']} MTg={meta['MTg']} T={meta['T']} "
          f"MEXP={meta['MEXP']} NR1={meta['NR1']}")
    if "sim" in sys.argv:
        n_it = int(sys.argv[sys.argv.index("sim") + 1]) if len(sys.argv) > 2 else 8
        import jax.numpy as jnp
        ni = np.asarray(jnp.zeros((N,), jnp.float32).at[jnp.asarray(inputs["in_indices"])].set(
            jnp.asarray(inputs["in_weights"], jnp.float32) * jnp.asarray(inputs["x"][0], jnp.float32)))
        b_in = (ni + inputs["biases"]).astype(np.float64)
        rw = inputs["rec_weights"].astype(np.float64)
        er, ec = inputs["edge_rows"], inputs["edge_cols"]
        yref = np.zeros(N, np.float64)
        for _ in range(n_it):
            s = np.bincount(er, weights=rw * yref[ec], minlength=N)
            v = s + b_in
            yref = np.where(v > 0.5, 1.0 - 0.25 / np.maximum(v, 0.5),
                            np.maximum(v, LEAK * v))
        scale = np.abs(yref).max()
        t0 = time.time()
        ys = _sim(cores, perm, meta, n_it, quant=False)
        print(f"sim(noquant,{n_it}) {time.time()-t0:.1f}s  max rel err:",
              np.abs(ys[perm] - yref).max() / scale)
        t0 = time.time()
        ysq = _sim(cores, perm, meta, n_it, quant=True)
        print(f"sim(fp16,{n_it}) {time.time()-t0:.1f}s  max rel err:",
              np.abs(ysq[perm] - yref).max() / scale)



# revision 42
# speedup vs baseline: 1.0008x; 1.0008x over previous
"""Bionetwork sparse-matvec recurrence on 8 trn2 NeuronCores.

y_{t+1} = act(A y_t + b_in), 150 iterations, A fixed sparse (3.2M edges,
100k nodes).  Dest-sharded across 8 cores; all routing tables SBUF-resident.

Per iteration, per core (local_scatter = vectorized GPSIMD within-row scatter):
  1. seed-scatter per dest-chunk g: canonical y -> run-starts of expansion
  2. segmented forward-fill via one tensor_tensor_scan (state=mask*state+seed)
  3. multiply by edge weights (fp16, in place)
  4. round-1 local_scatter: products -> staging tiles at col 128*t + dest_row
  5. PE transpose of each [128,128] staging tile (the cross-partition hop)
  6. round-2 local_scatter: transposed stream -> dest-slot layout
  7. segmented reduce straight into output order (per chunk: 15 64-wide
     slots for deg>32 dests + 18 32-wide slots; no fold pass)
  8. v = s + b_in; piecewise activation (select-fused); AllGather; reload y

Chunk assignment of dests is a greedy min-max balance (any dest fits any
chunk), flattening max edges per (chunk, src partition, dest row) -- that
max sets the staging-tile count T and the round-1/2 scatter cost.

Everything is table-driven; tables are built host-side from the (fixed)
edge lists and shipped as per-core input tensors to one shared program.
"""
import numpy as np

N = 100000
E = 3200000
P = 128
NCORES = 8
QW = 800                    # canonical width: 128*800 = 102400
NC_PAD = P * QW
SHARD = NC_PAD // NCORES    # 12800 = 128*100
KMAX = SHARD // P           # 100
ITERS = 150
LEAK = 0.01
RUN_CAP = 16                # fill rounds 1,2,4,8 cover runs of 16
SEED_REGIONS = 1
MAX_DST = 2046
TILES_PER_CALL = 15
SD = SEED_REGIONS * QW


def _ceil(a, b):
    return -(-a // b)


def _slot_of(c_src, d_core):
    """RDMA all-gather slot holding core c_src's shard on core d_core.

    Each core fires 7 single-dest relative remote_dma_broadcasts, slot x
    targeting Delta-tpb x.  Under the TRN2 physical-NC map (logical l -> real
    l for l<4, l^2 for l>=4) the slot-x write of sender s lands on core s^x
    for x<4 and s^x^2 for x>=4; inverting gives the receiver-side owner map.
    """
    delta = c_src ^ d_core
    return np.where(delta < 4, delta, delta ^ 2)


def _prep(x, in_weights, rec_weights, biases, out_weights,
          in_indices, edge_rows, edge_cols, out_indices):
    deg = np.bincount(edge_rows, minlength=N)
    assert deg.max() <= 64, f"max in-degree {deg.max()} > 64 unsupported"
    np2 = deg > 32  # wide dests get a 64-col slot, the rest a 32-col slot

    # Slot layout: every chunk holds NP2C 64-wide slots + NP1C 32-wide slots
    # per (core,row) bin, so ANY dest can be placed in ANY chunk.  A greedy
    # min-max pass then assigns dests to chunks to flatten the edge count per
    # (chunk, src partition, dest row) cell -- that max sets the staging tile
    # count T and with it the round-1/round-2 scatter cost.
    NP2C, NP1C = 15, 18
    NCH = 3
    CH = NP2C * 64 + NP1C * 32          # 1536
    FD = NCH * CH
    SLOTC = NP2C + NP1C                 # sp slots per chunk
    Kreal = KP = NCH * SLOTC            # 99
    assert Kreal <= KMAX
    NB = NCORES * P

    # deal dests round-robin over bins, np2 class first, LPT within class
    # (largest degree first improves the greedy min-max balance)
    rng = np.random.default_rng(12345)
    order = np.lexsort((rng.permutation(N), -deg, ~np2))
    i = np.arange(N)
    binid_pos = i % NB
    c_node = np.empty(N, np.int64)
    j_node = np.empty(N, np.int64)
    bin_node = np.empty(N, np.int64)
    c_node[order] = binid_pos % NCORES
    j_node[order] = binid_pos // NCORES
    bin_node[order] = binid_pos
    N2 = int(np2.sum())
    assert _ceil(N2, NB) <= NP2C * NCH and _ceil(N - N2, NB) <= NP1C * NCH
    rank_pos = np.empty(N, np.int64)
    rank_pos[order[:N2]] = np.arange(N2) // NB
    i1 = np.arange(N2, N)
    rank_pos[order[N2:]] = (i1 - N2 - ((i1 % NB) - N2) % NB) // NB
    r2max = int(rank_pos[order[:N2]].max()) + 1 if N2 else 0
    r1max = int(rank_pos[order[N2:]].max()) + 1

    # source-side fix: cap the per-(core,row,src-partition) edge TOTAL so the
    # dest-side chunk balance can land at MTg<=CAPT/3.  Swapping two
    # same-class nodes' dealing slots moves both their dest bin and their
    # source partition; accept swaps that lower the hot cell without pushing
    # any affected cell to the current max.  Delta-evaluated per candidate
    # (only edges incident to the swapped pair change cells).
    CAPT = 33
    rng2 = np.random.default_rng(777)
    ew_d, ew_s = edge_rows, edge_cols

    o_d = np.argsort(ew_d, kind="stable")
    dptr0 = np.searchsorted(ew_d[o_d], np.arange(N + 1))
    o_s = np.argsort(ew_s, kind="stable")
    sptr0 = np.searchsorted(ew_s[o_s], np.arange(N + 1))

    def _keys(eidx):
        return ((c_node[ew_d[eidx]] * P + j_node[ew_d[eidx]]) * P
                + j_node[ew_s[eidx]])

    def _tot():
        return np.bincount(_keys(np.arange(E)), minlength=NCORES * P * P)

    def _inc_edges(node):
        return np.concatenate([o_d[dptr0[node]:dptr0[node + 1]],
                               o_s[sptr0[node]:sptr0[node + 1]]])

    def _swap(a, bnode):
        for arr in (c_node, j_node, bin_node, rank_pos):
            arr[a], arr[bnode] = arr[bnode], arr[a]

    import time as _time
    tot = _tot()
    t_end = _time.time() + 60.0
    for _ in range(6000):
        cur = int(tot.max())
        if cur <= CAPT or _time.time() > t_end:
            break
        h = int(np.argmax(tot))
        hc, hj, hp = h // (P * P), (h // P) % P, h % P
        nodes_hj = np.flatnonzero((c_node == hc) & (j_node == hj))
        he = np.concatenate([o_d[dptr0[n]:dptr0[n + 1]] for n in nodes_hj])
        he = he[j_node[ew_s[he]] == hp]
        srcs, cnts = np.unique(ew_s[he], return_counts=True)
        s = int(srcs[np.argmax(cnts)])
        pool_t = np.flatnonzero(np2 == np2[s])
        es_ = _inc_edges(s)
        best = None
        for t in rng2.choice(pool_t, size=48, replace=False):
            t = int(t)
            if t == s:
                continue
            eidx = np.concatenate([es_, _inc_edges(t)])
            k_old = _keys(eidx)
            _swap(s, t)
            k_new = _keys(eidx)
            _swap(s, t)
            aff = np.unique(np.concatenate([k_old, k_new]))
            av = tot[aff].astype(np.int64)
            np.subtract.at(av, np.searchsorted(aff, k_old), 1)
            np.add.at(av, np.searchsorted(aff, k_new), 1)
            hpos = np.searchsorted(aff, h)
            if hpos >= aff.size or aff[hpos] != h or av[hpos] >= cur:
                continue
            nm = int(av.max())
            if nm >= cur:
                continue
            sc = (nm, int((av > CAPT).sum()))
            if best is None or sc < best[:2]:
                best = (nm, sc[1], t, k_old, k_new)
        if best is None:
            continue
        _, _, tbest, k_old, k_new = best
        eidx = np.concatenate([es_, _inc_edges(tbest)])
        np.subtract.at(tot, k_old, 1)
        _swap(s, tbest)
        np.add.at(tot, _keys(eidx), 1)
    # greedy chunk assignment, one round per (class, rank): each bin places
    # its rank-r dest into the chunk minimizing that bin-row's max cell
    p0_of_node = j_node
    e_b = bin_node[edge_rows]
    e_p0 = p0_of_node[edge_cols]
    e_key = np.where(np2[edge_rows], 0, 256) + rank_pos[edge_rows]
    eo = np.argsort(e_key, kind="stable")
    e_key_s = e_key[eo]
    n_key = np.where(np2, 0, 256) + rank_pos
    no = np.argsort(n_key, kind="stable")
    n_key_s = n_key[no]

    cells = np.zeros((NCH, NB, P), np.int32)
    cnt2 = np.zeros((NCH, NB), np.int32)
    cnt1 = np.zeros((NCH, NB), np.int32)
    band_of = np.zeros(N, np.int64)
    kloc_of = np.zeros(N, np.int64)

    def _rounds(base_key, nmax, cnt, cap):
        for r in range(nmax):
            key = base_key + r
            na, nb_ = np.searchsorted(n_key_s, [key, key + 1])
            if na == nb_:
                continue
            nodes_r = no[na:nb_]
            ea, ebnd = np.searchsorted(e_key_s, [key, key + 1])
            H = np.zeros((NB, P), np.int32)
            if ea < ebnd:
                es = eo[ea:ebnd]
                np.add.at(H, (e_b[es], e_p0[es]), 1)
            cmax = (cells + H[None]).max(axis=2) * 64 + cnt
            cmax[cnt >= cap] = 1 << 30
            band = np.argmin(cmax, axis=0)
            bsel = bin_node[nodes_r]
            bb = band[bsel]
            band_of[nodes_r] = bb
            kloc_of[nodes_r] = cnt[bb, bsel]
            for ch in range(NCH):
                m = bsel[bb == ch]
                cells[ch, m] += H[m]
                cnt[ch, m] += 1

    _rounds(0, r2max, cnt2, NP2C)
    _rounds(256, r1max, cnt1, NP1C)

    # swap-refinement: the greedy's per-chunk max cells are online-placement
    # flukes; swapping two same-class dests of one bin between chunks just
    # exchanges their (chunk, slot) pairs, so descend on the hot cells
    eds = np.argsort(edge_rows, kind="stable")
    er_s = edge_rows[eds]
    dptr = np.searchsorted(er_s, np.arange(N + 1))
    e_p0s = e_p0[eds]

    def _hist(d):
        return np.bincount(e_p0s[dptr[d]:dptr[d + 1]], minlength=P)

    bysort = np.lexsort((np.arange(N), bin_node))
    bptr = np.searchsorted(bin_node[bysort], np.arange(NB + 1))
    t_end2 = _time.time() + 60.0
    stall = 0
    while True:
        m = int(cells.max())
        if m <= 11 or _time.time() > t_end2 or stall > 30:
            break
        hot = np.argwhere(cells == m)
        progressed = False
        for ch, bin_, p0h in hot[:16]:
            ch = int(ch)
            dl = bysort[bptr[bin_]:bptr[bin_ + 1]]
            mine = dl[band_of[dl] == ch]
            hists = {int(d): _hist(int(d)) for d in dl}
            best = None
            for d in mine:
                hd = hists[int(d)]
                if hd[p0h] == 0:
                    continue
                for d2 in dl[(band_of[dl] != ch) & (np2[dl] == np2[d])]:
                    ch2 = int(band_of[d2])
                    h2 = hists[int(d2)]
                    na = int((cells[ch, bin_] - hd + h2).max())
                    nb_ = int((cells[ch2, bin_] - h2 + hd).max())
                    # plateau swaps (nb_ == m) allowed: they move the hot
                    # cell even when a strict improvement doesn't exist
                    if na < m and nb_ <= m:
                        sc = (max(na, nb_), na + nb_)
                        if best is None or sc < best[:2]:
                            best = (sc[0], sc[1], int(d), d2, ch2)
            if best is not None:
                _, _, d, d2, ch2 = best
                hd, h2 = hists[d], hists[int(d2)]
                cells[ch, bin_] += h2 - hd
                cells[ch2, bin_] += hd - h2
                band_of[d], band_of[d2] = ch2, ch
                kloc_of[d], kloc_of[d2] = kloc_of[d2], kloc_of[d]
                progressed = True
        stall = 0 if progressed else stall + 1

    k_node = band_of * SLOTC + np.where(np2, kloc_of, NP2C + kloc_of)
    perm = SHARD * c_node + KMAX * j_node + k_node

    # per-edge slot column: rank within dest (stable edge order)
    so = np.argsort(edge_rows, kind="stable")
    ds = edge_rows[so]
    st = np.r_[0, np.flatnonzero(np.diff(ds)) + 1]
    sid = np.zeros(E, np.int64)
    sid[st[1:]] = 1
    sid = np.cumsum(sid)
    e_drank = np.empty(E, np.int64)
    e_drank[so] = np.arange(E) - st[sid]
    e_np2 = np2[edge_rows]
    e_kloc = kloc_of[edge_rows]
    f_local = np.where(e_np2, 64 * e_kloc + e_drank,
                       NP2C * 64 + 32 * e_kloc + e_drank)
    fglob = band_of[edge_rows] * CH + f_local

    import jax.numpy as jnp
    node_in = np.asarray(
        jnp.zeros((N,), jnp.float32).at[jnp.asarray(in_indices)].set(
            jnp.asarray(in_weights, jnp.float32) * jnp.asarray(x[0], jnp.float32)))
    b_in_full = node_in + biases.astype(np.float32)

    dnew, snew = perm[edge_rows], perm[edge_cols]
    w_all = rec_weights.astype(np.float32)
    dcore = dnew // SHARD

    # ---------- pass 1: per-core edge geometry ----------
    # per-core y layout: y_parts[j, 100*slot_of(c_src, core) + k] holds the
    # (c_src, j, k) shard entry (each slot is the sender's y16 tile verbatim)
    geo = []
    for c in range(NCORES):
        em = np.where(dcore == c)[0]
        d_loc = dnew[em] - SHARD * c
        j, k = d_loc // KMAX, d_loc % KMAX
        s_new = snew[em]
        s_c, s_jk = s_new // SHARD, s_new % SHARD
        p0 = s_jk // KMAX
        q0 = KMAX * _slot_of(s_c, c) + s_jk % KMAX
        w = w_all[em]
        ne = em.size

        def ranks_of(key):
            so = np.argsort(key, kind="stable")
            ks = key[so]
            st = np.r_[0, np.flatnonzero(np.diff(ks)) + 1]
            sid = np.zeros(ne, np.int64)
            sid[st[1:]] = 1
            sid = np.cumsum(sid)
            r = np.arange(ne) - st[sid]
            out = np.empty(ne, np.int64)
            out[so] = r
            return out

        f = fglob[em]
        g = f // CH
        trank = ranks_of((g * P + p0) * P + j)
        # expansion position within (g,p0) ordered by q0, and rank within source
        so3 = np.lexsort((q0, p0, g))
        gp = (g * P + p0)[so3]
        st = np.r_[0, np.flatnonzero(np.diff(gp)) + 1]
        sid = np.zeros(ne, np.int64)
        sid[st[1:]] = 1
        sid = np.cumsum(sid)
        m_pos = np.empty(ne, np.int64)
        m_pos[so3] = np.arange(ne) - st[sid]
        gpq = ((g * P + p0) * QW + q0)[so3]
        st4 = np.r_[0, np.flatnonzero(np.diff(gpq)) + 1]
        sid4 = np.zeros(ne, np.int64)
        sid4[st4[1:]] = 1
        sid4 = np.cumsum(sid4)
        src_rank = np.empty(ne, np.int64)
        src_rank[so3] = np.arange(ne) - st4[sid4]
        geo.append(dict(j=j, p0=p0, q0=q0, w=w, f=f, g=g,
                        trank=trank, m_pos=m_pos, src_rank=src_rank, ne=ne))

    # uniform per-chunk sizes across cores
    M1 = np.zeros(NCH, np.int64)
    MTg = np.zeros(NCH, np.int64)
    for gg in geo:
        for g2 in range(NCH):
            sel = gg["g"] == g2
            if sel.any():
                M1[g2] = max(M1[g2], int(gg["m_pos"][sel].max()) + 1)
                MTg[g2] = max(MTg[g2], int(gg["trank"][sel].max()) + 1)
    M1 = (_ceil(M1, 2) * 2).astype(np.int64)
    EB = np.r_[0, np.cumsum(M1)]         # expansion bases
    MEXP = int(EB[-1])
    TBASE = np.r_[0, np.cumsum(MTg)]     # tile bases
    T = int(TBASE[-1])
    # round-1 call structure: (g, t0, t1), evenly-split windows <= 15 tiles
    r1_struct = []
    for g2 in range(NCH):
        tg = int(MTg[g2])
        ncall = _ceil(tg, TILES_PER_CALL)
        base, rem = divmod(tg, ncall)
        t0 = 0
        for ci in range(ncall):
            nt = base + (1 if ci < rem else 0)
            r1_struct.append((g2, t0, t0 + nt))
            t0 += nt
    NR1 = len(r1_struct)

    # ---------- pass 2: tables ----------
    cores = []
    for c in range(NCORES):
        gg = geo[c]
        j, p0, q0, w = gg["j"], gg["p0"], gg["q0"], gg["w"]
        f, g, trank, m_pos, src_rank = (gg["f"], gg["g"], gg["trank"],
                                        gg["m_pos"], gg["src_rank"])
        m_glob = EB[g] + m_pos
        dist = src_rank

        seedidx = np.full((NCH, P, SD), -1, np.int16)
        sm = dist == 0
        seedidx[g[sm], p0[sm], q0[sm]] = m_pos[sm].astype(np.int16)

        # scan fill mask: 1.0 inside a source run (copy state), 0.0 at starts
        runmask = np.zeros((P, MEXP), np.float16)
        mm = dist > 0
        runmask[p0[mm], m_glob[mm]] = 1.0

        w_exp = np.zeros((P, MEXP), np.float16)
        w_exp[p0, m_glob] = w.astype(np.float16)

        idx1 = []
        for (g2, t0, t1) in r1_struct:
            sel = (g == g2) & (trank >= t0) & (trank < t1)
            idx = np.full((P, int(M1[g2])), -1, np.int16)
            idx[p0[sel], m_pos[sel]] = (128 * (trank[sel] - t0) + j[sel]).astype(np.int16)
            idx1.append(idx)

        idx2 = []
        for g2 in range(NCH):
            sel = g == g2
            idx = np.full((P, 128 * int(MTg[g2])), -1, np.int16)
            idx[j[sel], 128 * trank[sel] + p0[sel]] = (f[sel] - g2 * CH).astype(np.int16)
            idx2.append(idx)

        b_in_t = np.zeros((P, Kreal), np.float32)
        nid = np.where((perm >= SHARD * c) & (perm < SHARD * (c + 1)))[0]
        dl = perm[nid] - SHARD * c
        b_in_t[dl // KMAX, dl % KMAX] = b_in_full[nid]

        cores.append(dict(seedidx=seedidx, runmask=runmask, w_exp=w_exp,
                          idx1=idx1, idx2=idx2, b_in_t=b_in_t))

    meta = dict(Kreal=Kreal, KP=KP, FD=FD, NCH=NCH, CH=CH, M1=M1, EB=EB,
                MTg=MTg, TBASE=TBASE, T=T, MEXP=MEXP, NR1=NR1,
                r1_struct=r1_struct, NP2C=NP2C, NP1C=NP1C, SLOTC=SLOTC)
    return cores, perm, meta


def _act_np(v):
    y1 = np.maximum(v, np.float32(LEAK) * v)
    ysat = (1.0 - 0.25 / np.maximum(v, 0.5)).astype(v.dtype)
    return np.where(v > 0.5, ysat, y1)


def _sim(cores, perm, meta, n_iters, quant=True):
    dt = np.float16 if quant else np.float32
    Kreal, KP, FD, NCH, CH = (meta["Kreal"], meta["KP"], meta["FD"],
                              meta["NCH"], meta["CH"])
    M1, EB, MTg, TBASE, T, MEXP = (meta["M1"], meta["EB"], meta["MTg"],
                                   meta["TBASE"], meta["T"], meta["MEXP"])
    y = np.zeros(NC_PAD, np.float32)
    jj_, kk_ = np.meshgrid(np.arange(P), np.arange(KMAX), indexing="ij")
    for it in range(n_iters):
        y_next = np.zeros(NC_PAD, np.float32)
        for c, tb in enumerate(cores):
            # per-core y_parts layout: slot x holds the shard of owner(c, x)
            seed_data = np.zeros((P, QW), dt)
            for x in range(NCORES):
                own = (c ^ x) if x < 4 else (c ^ x ^ 2)
                seed_data[:, KMAX * x:KMAX * (x + 1)] = y[
                    SHARD * own + KMAX * jj_ + kk_].astype(dt)
            seeds = np.zeros((P, MEXP), dt)
            for g2 in range(NCH):
                sidx = tb["seedidx"][g2]
                pp, cc = np.where(sidx >= 0)
                seeds[pp, EB[g2] + sidx[pp, cc]] = seed_data[pp, cc]
            # segmented forward-fill scan: state = mask*state + seed (fp32
            # state, downcast per element) per chunk
            exp_t = np.zeros((P, MEXP), dt)
            rm = tb["runmask"].astype(np.float32)
            sd32 = seeds.astype(np.float32)
            for g2 in range(NCH):
                st = np.zeros(P, np.float32)
                for t in range(int(EB[g2]), int(EB[g2 + 1])):
                    st = rm[:, t] * st + sd32[:, t]
                    exp_t[:, t] = st.astype(dt)
            prod = (exp_t.astype(np.float32) * tb["w_exp"].astype(np.float32)).astype(dt)
            staging = np.zeros((P, 128 * T), dt)
            for ci, (g2, t0, t1) in enumerate(meta["r1_struct"]):
                idx = tb["idx1"][ci]
                data = prod[:, EB[g2]:EB[g2] + M1[g2]]
                pp, cc = np.where(idx >= 0)
                staging[pp, 128 * (TBASE[g2] + t0) + idx[pp, cc]] = data[pp, cc]
            t2 = np.zeros_like(staging)
            for t in range(T):
                t2[:, 128 * t:128 * (t + 1)] = staging[:, 128 * t:128 * (t + 1)].T
            slots = np.zeros((P, FD), dt)
            for g2 in range(NCH):
                idx = tb["idx2"][g2]
                data = t2[:, 128 * TBASE[g2]:128 * (TBASE[g2] + MTg[g2])]
                pp, cc = np.where(idx >= 0)
                slots[pp, g2 * CH + idx[pp, cc]] = data[pp, cc]
            NP2C, NP1C, SLOTC = meta["NP2C"], meta["NP1C"], meta["SLOTC"]
            sp = np.zeros((P, KP), np.float32)
            for g2 in range(NCH):
                ch = slots[:, g2 * CH:(g2 + 1) * CH].astype(np.float32)
                w2 = ch[:, :NP2C * 64].reshape(P, NP2C, 64).sum(axis=2)
                w1 = ch[:, NP2C * 64:].reshape(P, NP1C, 32).sum(axis=2)
                c0 = g2 * SLOTC
                sp[:, c0:c0 + NP2C] = w2
                sp[:, c0 + NP2C:c0 + SLOTC] = w1
            s = sp.astype(dt).astype(np.float32)[:, :Kreal]
            v = s + tb["b_in_t"]
            y32 = _act_np(v)
            jj, kk2 = np.meshgrid(np.arange(P), np.arange(Kreal), indexing="ij")
            y_next[SHARD * c + KMAX * jj.ravel() + kk2.ravel()] = y32.ravel()
        y = y_next
    return y


# ============================ BASS KERNEL ============================

def _build(cores, meta, n_iters, no_cc=False, skip_last_exchange=True):
    import concourse.bacc as bacc
    import concourse.mybir as mybir
    import concourse.tile as tile
    from concourse.masks import make_identity

    f16, f32, i16 = mybir.dt.float16, mybir.dt.float32, mybir.dt.int16
    AOP = mybir.AluOpType
    Kreal, KP, FD, NCH, CH = (meta["Kreal"], meta["KP"], meta["FD"],
                              meta["NCH"], meta["CH"])
    M1, EB, MTg, TBASE, T, MEXP, NR1 = (meta["M1"], meta["EB"], meta["MTg"],
                                        meta["TBASE"], meta["T"],
                                        meta["MEXP"], meta["NR1"])
    NP2C, NP1C, SLOTC = meta["NP2C"], meta["NP1C"], meta["SLOTC"]
    DSTW = [min(FD, (g + 1) * CH) - g * CH for g in range(NCH)]

    nc = bacc.Bacc("TRN2", target_bir_lowering=False, num_swdge_queues=2)

    d_seed = [nc.dram_tensor(f"t_seed{g}", [P, SD], i16, kind="ExternalInput")
              for g in range(NCH)]
    d_rmask = nc.dram_tensor("t_rmask", [P, MEXP], f16, kind="ExternalInput")
    d_wexp = nc.dram_tensor("t_wexp", [P, MEXP], f16, kind="ExternalInput")
    d_idx1 = [nc.dram_tensor(f"t_idx1_{ci}", [P, int(M1[g2])], i16,
                             kind="ExternalInput")
              for ci, (g2, _, _) in enumerate(meta["r1_struct"])]
    d_idx2 = [nc.dram_tensor(f"t_idx2_{g}", [P, 128 * int(MTg[g])], i16,
                             kind="ExternalInput") for g in range(NCH)]
    d_bin = nc.dram_tensor("t_bin", [P, Kreal], f32, kind="ExternalInput")
    d_yout = nc.dram_tensor("y_out", [P, Kreal], f16, kind="ExternalOutput")

    # cross-core y exchange: 7 single-dest relative remote_dma_broadcasts per
    # iteration land each core's y16 in the peers' y_parts slots (slot order
    # [own,^1,^2,^3,^6,^7,^4,^5] = _slot_of under the Q7 lane map); rsem
    # counts arrivals (2/send), lsem send completions (16/send)
    rsem = nc.alloc_semaphore("rdma_rsem")
    lsem = nc.alloc_semaphore("rdma_lsem")
    patches = {}  # inst name -> (sem num, wait value); placeholder 0 -> value

    def _gate(inst, sem, val):
        inst.wait_op(sem, 0, "sem-ge")
        patches[inst.ins.name] = (sem.num, val)
        return inst

    with tile.TileContext(nc) as tc:
        with tc.tile_pool(name="tables", bufs=1) as tp, \
             tc.tile_pool(name="psum", bufs=8, space="PSUM") as pp:
            t_seed = [tp.tile([P, SD], i16, name=f"seed{g}") for g in range(NCH)]
            t_rmask = tp.tile([P, MEXP], f16, name="rmask")
            t_wexp = tp.tile([P, MEXP], f16, name="wexp")
            t_idx1 = [tp.tile([P, int(M1[g2])], i16, name=f"i1_{ci}")
                      for ci, (g2, _, _) in enumerate(meta["r1_struct"])]
            t_idx2 = [tp.tile([P, 128 * int(MTg[g])], i16, name=f"i2_{g}")
                      for g in range(NCH)]
            t_bin = tp.tile([P, Kreal], f32, name="bin")
            ident = tp.tile([P, P], f16, name="ident")
            ypA = tp.tile([P, QW], f16, name="ypA")
            ypB = tp.tile([P, QW], f16, name="ypB")
            ttok = tp.tile([1, 2], f16, name="ttok")
            expb = [tp.tile([P, int(M1[g])], f16, name=f"expb{g}")
                    for g in range(NCH)]
            seedb = [tp.tile([P, int(M1[g])], f16, name=f"seedb{g}")
                     for g in range(NCH)]
            stag = [tp.tile([P, 128 * int(MTg[g])], f16, name=f"stag{g}")
                    for g in range(NCH)]
            t2d = [tp.tile([P, 128 * int(MTg[g])], f16, name=f"t2d{g}")
                   for g in range(NCH)]
            slots = [tp.tile([P, DSTW[g]], f16, name=f"slots{g}")
                     for g in range(NCH)]
            sp = tp.tile([P, KP], f16, name="sp")
            vv = tp.tile([P, Kreal], f32, name="vv")
            y1b = tp.tile([P, Kreal], f32, name="y1b")
            rb = tp.tile([P, Kreal], f32, name="rb")
            y16 = tp.tile([P, KMAX], f16, name="y16")

            for g in range(NCH):
                nc.sync.dma_start(t_seed[g][:], d_seed[g][:])
                nc.sync.dma_start(t_idx2[g][:], d_idx2[g][:])
            nc.sync.dma_start(t_rmask[:], d_rmask[:])
            for ci in range(NR1):
                nc.sync.dma_start(t_idx1[ci][:], d_idx1[ci][:])
            nc.sync.dma_start(t_wexp[:], d_wexp[:])
            nc.sync.dma_start(t_bin[:], d_bin[:])
            make_identity(nc, ident[:])
            nc.vector.memset(ypA[:], 0.0)
            nc.vector.memset(y16[:], 0.0)

            r1_by_g = {}
            for ci, (g2, t0, t1) in enumerate(meta["r1_struct"]):
                r1_by_g.setdefault(g2, []).append((ci, t0, t1))

            # small chunk last: its short r1->copy->r2 chain ends the iteration
            g_order = [0, 1, 2] if NCH == 3 else list(range(NCH))

            def body(it, last=False):
                buf_r = ypA if it % 2 == 0 else ypB
                buf_w = ypB if it % 2 == 0 else ypA
                # phase 1: seeds, scan-fill, weight mult, round-1 scatters --
                # all chunks' r1 calls queue on Pool ahead of any r2, so the
                # last chunk's staging is ready before Pool reaches its r2
                for g in g_order:
                    w0, w1 = int(EB[g]), int(EB[g + 1])
                    mw = int(M1[g])
                    sc = nc.gpsimd.local_scatter(
                        seedb[g][:], buf_r[:], t_seed[g][:],
                        channels=P, num_elems=mw, num_idxs=SD)
                    if it > 0:
                        _gate(sc, rsem, 14 * it)
                    nc.vector.tensor_tensor_scan(
                        expb[g][:], t_rmask[:, w0:w1], seedb[g][:], 0.0,
                        op0=AOP.mult, op1=AOP.add)
                    nc.vector.tensor_tensor(expb[g][:], expb[g][:],
                                            t_wexp[:, w0:w1], op=AOP.mult)
                    for ci, t0, t1 in r1_by_g[g]:
                        nt = t1 - t0
                        nc.gpsimd.local_scatter(
                            stag[g][:, 128 * t0:128 * t1], expb[g][:],
                            t_idx1[ci][:], channels=P, num_elems=128 * nt,
                            num_idxs=mw)

                # phase 2: transposes; PSUM->SBUF copies alternate DVE/Act
                nbatch = 0
                for g in g_order:
                    Tg = int(MTg[g])
                    for tb0 in range(0, Tg, 8):
                        nb = min(8, Tg - tb0)
                        pt = pp.tile([P, 8 * P], f16, space="PSUM", tag="tr",
                                     name="tr")
                        for t in range(tb0, tb0 + nb):
                            nc.tensor.transpose(
                                pt[:, 128 * (t - tb0):128 * (t - tb0 + 1)],
                                stag[g][:, 128 * t:128 * (t + 1)], ident[:])
                        dst = t2d[g][:, 128 * tb0:128 * (tb0 + nb)]
                        if nbatch % 2 == 0:
                            nc.vector.tensor_copy(dst, pt[:, 0:128 * nb])
                        else:
                            nc.scalar.copy(dst, pt[:, 0:128 * nb])
                        nbatch += 1
                # phase 3: round-2 scatters + segmented reduces (64-wide
                # slots for wide dests then 32-wide; fp16 sums of |w*y|<0.2
                # stay O(1), validated against an fp64 reference)
                for g in g_order:
                    nc.gpsimd.local_scatter(
                        slots[g][:], t2d[g][:],
                        t_idx2[g][:], channels=P, num_elems=DSTW[g],
                        num_idxs=128 * int(MTg[g]))
                    c0 = g * SLOTC
                    n2w = NP2C * 64
                    with nc.allow_low_precision(reason="fp16 slot sums"):
                        nc.vector.tensor_reduce(
                            sp[:, c0:c0 + NP2C],
                            slots[g][:, 0:n2w].rearrange(
                                "p (k s) -> p k s", s=64),
                            axis=mybir.AxisListType.X, op=AOP.add)
                        nc.vector.tensor_reduce(
                            sp[:, c0 + NP2C:c0 + SLOTC],
                            slots[g][:, n2w:CH].rearrange(
                                "p (k s) -> p k s", s=32),
                            axis=mybir.AxisListType.X, op=AOP.add)
                # piecewise activation == min(leaky, saturating): for v<=0.5
                # the clamp makes 1-0.25/max(v,0.5) = 0.5 >= leaky(v); for
                # v>0.5 the saturating branch is always below v.  Sliced in
                # two: cols [0:66] (chunks 1,0, reduced early) run under the
                # last chunk's r2; only 33 cols remain on the tail chain.
                for a0, a1 in ((0, 2 * SLOTC), (2 * SLOTC, Kreal)):
                    nc.vector.tensor_tensor(vv[:, a0:a1], sp[:, a0:a1],
                                            t_bin[:, a0:a1], op=AOP.add)
                    nc.vector.scalar_tensor_tensor(
                        y1b[:, a0:a1], vv[:, a0:a1], float(LEAK), vv[:, a0:a1],
                        op0=AOP.mult, op1=AOP.max)
                    nc.vector.tensor_scalar_max(rb[:, a0:a1], vv[:, a0:a1], 0.5)
                    nc.vector.reciprocal(rb[:, a0:a1], rb[:, a0:a1])
                    nc.vector.tensor_scalar(rb[:, a0:a1], rb[:, a0:a1],
                                            -0.25, 1.0,
                                            op0=AOP.mult, op1=AOP.add)
                    ymin = nc.vector.tensor_tensor(y16[:, a0:a1], y1b[:, a0:a1],
                                                   rb[:, a0:a1], op=AOP.min)
                    if it > 0:
                        # previous round's sends must have finished reading y16
                        _gate(ymin, lsem, 112 * it)
                if last:
                    return  # final shard never leaves this core pre-gather
                # fire the prepared sends; the signals_writable token orders
                # next round's seeds after the trigger so their rsem gate is
                # reachable.  The local slot-0 copy comes AFTER the trigger:
                # tile's whole-tile dep granularity would otherwise chain
                # desc-gen behind copy behind activation.
                nc.gpsimd.trigger_dma(count=None,
                                      signals_writable=[buf_w[:, 0:2]])
                nc.scalar.copy(buf_w[:, 0:KMAX], y16[:])

            for it in range(n_iters):
                body(it, last=(skip_last_exchange and it == n_iters - 1))
            nc.sync.dma_start(d_yout[:], y16[:, 0:Kreal])

    # patch the placeholder cross-core gates to their real per-round values
    # (a real value would deadlock tile's single-core scheduling simulator)
    n_patched = 0
    for bb in nc.m.functions[0].blocks:
        for ins in bb.instructions:
            tgt = patches.get(ins.name)
            if tgt is None:
                continue
            si = ins.sync_info
            for w in (si.on_wait if si else []):
                if w.id == tgt[0] and w.wait_mode == "sem-ge-imm" \
                        and w.wait_value == 0:
                    w.wait_value = tgt[1]
                    n_patched += 1
    assert n_patched == len(patches), (n_patched, len(patches))

    nc.compile()
    return nc


def _in_maps(cores, meta):
    maps = []
    for tb in cores:
        m = {"t_wexp": tb["w_exp"], "t_bin": tb["b_in_t"],
             "t_rmask": tb["runmask"]}
        for g in range(meta["NCH"]):
            m[f"t_seed{g}"] = tb["seedidx"][g]
            m[f"t_idx2_{g}"] = tb["idx2"][g]
        for ci in range(meta["NR1"]):
            m[f"t_idx1_{ci}"] = tb["idx1"][ci]
        maps.append(m)
    return maps


def make_timing_cost_model():
    """Cost model for single-core TimelineSim (no_exec) runs of this kernel.

    The stock rust model has a known gap: in no_exec mode nobody enqueues
    remote-DMA prep entries into the SWDGE FIFO, so InstTriggerDma models no
    transfer time and no sem delivery, and the cross-core gates deadlock.
    This shim enqueues the prep entries exactly as the executor would (the
    rust trigger visit then applies its own transfer/ack formulas), and
    mirrors each incoming remote-sem delivery locally: in symmetric SPMD the
    peers' sends to this core complete at the same relative times as this
    core's sends to them.
    """
    import concourse.bass_isa as bass_isa
    import concourse.mybir as mb
    from concourse.cost_model import InstructionCostModel
    from concourse.cost_model_rust import (
        SemUpdate as EvSemUpdate, RemoteSemUpdate as EvRemoteSemUpdate)
    from concourse.dge_state import SwdgePrepEntry, return_none
    from concourse.hw_specs import get_hw_spec

    class RdmaCostModel(InstructionCostModel):
        def __init__(self, hw_spec):
            super().__init__(hw_spec)
            self._pending_mirror = {}  # queue_num -> [(sem_id, inc, n_dests)]

        def visit(self, instruction, sim):
            if sim.instruction_executor is not None:
                return super().visit(instruction, sim)
            if isinstance(instruction, bass_isa.InstRemoteDMABroadcastDescs):
                fifo = sim.swdge[instruction.queue_num]
                grp, ndescs = fifo.start_broadcast_group(instruction)
                fifo.await_space(SwdgePrepEntry(
                    instruction, ndescs, ndescs, instruction.local_sem_update,
                    return_none, grp))
                n_dests = len([d for d in instruction.dests if d >= 0])
                self._pending_mirror.setdefault(instruction.queue_num, []).append(
                    (instruction.remote_sem, 16 // len(instruction.dests),
                     n_dests))
            timelines = super().visit(instruction, sim)
            if isinstance(instruction, bass_isa.InstTriggerDma):
                pend = self._pending_mirror.get(instruction.queue_num, [])
                if pend:
                    out = []
                    for ev in timelines[0]:
                        out.append(ev)
                        if isinstance(ev, EvRemoteSemUpdate) and pend:
                            sem_id, inc, n_dests = pend[0]
                            out.append(EvSemUpdate(mb.SyncUpdate(
                                sync_type="semaphore", id=sem_id,
                                update_mode="sem-add-imm", update_value=inc)))
                            if n_dests <= 1:
                                pend.pop(0)
                            else:
                                pend[0] = (sem_id, inc, n_dests - 1)
                    timelines[0] = out
            return timelines

    return RdmaCostModel(get_hw_spec("TRN2"))


def _gather_y(res, meta):
    Kreal = meta["Kreal"]
    y_full = np.zeros(NC_PAD, np.float32)
    jj, kk2 = np.meshgrid(np.arange(P), np.arange(Kreal), indexing="ij")
    for c in range(NCORES):
        y32 = res.results[c]["y_out"]
        y_full[SHARD * c + KMAX * jj.ravel() + kk2.ravel()] = y32.ravel()
    return y_full


def kernel(**inputs):
    from concourse.bass_utils import run_bass_kernel_spmd
    inputs = {k: np.asarray(v) for k, v in inputs.items()}
    cores, perm, meta = _prep(**inputs)
    nc = _build(cores, meta, ITERS)
    maps = _in_maps(cores, meta)
    res = run_bass_kernel_spmd(nc, maps, core_ids=list(range(NCORES)))
    y_old = _gather_y(res, meta)[perm]
    out = (inputs["out_weights"].astype(np.float32)
           * y_old[inputs["out_indices"]])[None, :]
    return out.astype(np.float32)


if __name__ == "__main__":
    import sys, time
    sys.path.insert(0, "/root/problem")
    import reference
    inputs = {k: np.asarray(v) for k, v in reference.setup_inputs().items()}
    t0 = time.time()
    cores, perm, meta = _prep(**inputs)
    print(f"prep {time.time()-t0:.1f}s Kreal={meta['Kreal']} KP={meta['KP']} "
          f"FD={meta['FD']} M1={meta['M1']} MTg={meta['MTg']} T={meta['T']} "
          f"MEXP={meta['MEXP']} NR1={meta['NR1']}")
    if "sim" in sys.argv:
        n_it = int(sys.argv[sys.argv.index("sim") + 1]) if len(sys.argv) > 2 else 8
        import jax.numpy as jnp
        ni = np.asarray(jnp.zeros((N,), jnp.float32).at[jnp.asarray(inputs["in_indices"])].set(
            jnp.asarray(inputs["in_weights"], jnp.float32) * jnp.asarray(inputs["x"][0], jnp.float32)))
        b_in = (ni + inputs["biases"]).astype(np.float64)
        rw = inputs["rec_weights"].astype(np.float64)
        er, ec = inputs["edge_rows"], inputs["edge_cols"]
        yref = np.zeros(N, np.float64)
        for _ in range(n_it):
            s = np.bincount(er, weights=rw * yref[ec], minlength=N)
            v = s + b_in
            yref = np.where(v > 0.5, 1.0 - 0.25 / np.maximum(v, 0.5),
                            np.maximum(v, LEAK * v))
        scale = np.abs(yref).max()
        t0 = time.time()
        ys = _sim(cores, perm, meta, n_it, quant=False)
        print(f"sim(noquant,{n_it}) {time.time()-t0:.1f}s  max rel err:",
              np.abs(ys[perm] - yref).max() / scale)
        t0 = time.time()
        ysq = _sim(cores, perm, meta, n_it, quant=True)
        print(f"sim(fp16,{n_it}) {time.time()-t0:.1f}s  max rel err:",
              np.abs(ysq[perm] - yref).max() / scale)

